# revision 1
# baseline (speedup 1.0000x reference)
"""Trainium2 Bass kernel for bidirectional gated linear recurrence block.

Reference computation (per spatial position, channel-mixing MLPs):
  Z = tanh(W_z2 @ tanh(W_z1 @ x + b_z1) + b_z2)
  F = sigmoid(W_f2 @ tanh(W_f1 @ x + b_f1) + b_f2)
  channels 0:32  : h_t = F*h_{t-1} + (1-F)*Z forward over T
  channels 32:64 : same recurrence backward over T

Sharding: H (=64) split across 8 cores, 8 rows each. Everything else is
per-position so no collectives are needed.

Per-core dataflow (all fp32):
  - tiles of (b, h-pair): x_tile [64c, t=32, s=128] where s=(h2,w), DMA with
    512B-contiguous DRAM runs.
  - L1 matmul per 512-position block: lhsT = [Wz1;Wf1]^T [64,128],
    rhs = x strided (s-outer, t-inner) -> PSUM [128, 512] (h1z | h1f).
  - tanh via ScalarE (bias folded) -> h1 SBUF.
  - L2: col-tiled matmuls pack TWO blocks (P,Q) onto 128 PSUM partitions:
    psZ rows = [zP_fwd, zQ_fwd, zP_bwd, zQ_bwd] (32 chans each), psF same
    for the gate. Gate branch pre-scaled by 0.5 so one tanh pass gives
    t = tanh(m/2); then f = 0.5+0.5t, 1-f = 0.5-0.5t.
  - DVE: a = 0.5u+0.5 (zeroed at each segment's first step), g=(0.5-0.5u)*z,
    y = tensor_tensor_scan(a, g) along (s,t) free dim; bwd rows use
    negative-stride APs so the same forward scan implements reversed time.
  - DMA out per 32-row slice.
"""

import numpy as np

B, C, T, H, W = 2, 64, 32, 64, 64
NCORES = 8
HL = H // NCORES          # 8 h-rows per core
HP = HL // 2              # 4 h-pair tiles per (b)
S = 2 * W                 # 128 positions per h-pair (h2, w)
NBLK = S // 16            # 8 blocks of 512 (=16 s * 32 t) per tile
CH = C // 2               # 32 = fwd (or bwd) channel count

_built = {}


def _build():
    import concourse.bass as bass
    import concourse.mybir as mybir
    import concourse.tile as tile
    from concourse import bacc

    fp32 = mybir.dt.float32
    f32r = mybir.dt.float32r
    nc = bacc.Bacc(None, target_bir_lowering=False)

    x = nc.dram_tensor("x", [B, C, T, HL, W], mybir.dt.float32r, kind="ExternalInput")
    w1catT = nc.dram_tensor("w1catT", [C, 2 * C], mybir.dt.float32r, kind="ExternalInput")
    w2blk = nc.dram_tensor("w2blk", [2 * C, 2 * C], mybir.dt.bfloat16, kind="ExternalInput")
    b1cat = nc.dram_tensor("b1cat", [2 * C, 1], fp32, kind="ExternalInput")
    bz2p = nc.dram_tensor("bz2p", [2 * C, 1], fp32, kind="ExternalInput")
    bf2p = nc.dram_tensor("bf2p", [2 * C, 1], fp32, kind="ExternalInput")
    y = nc.dram_tensor("y", [B, C, T, HL, W], fp32, kind="ExternalOutput")

    MUL = mybir.AluOpType.mult
    ADD = mybir.AluOpType.add
    TANH = mybir.ActivationFunctionType.Tanh

    def rev2d(ap2d):
        # Reverse the free dim of a 2D [P, F] contiguous AP (step 1 -> -1).
        (pstep, pcnt), (fstep, fcnt) = [list(d) for d in ap2d.ap]
        assert fstep == 1, ap2d.ap
        return bass.AP(
            tensor=ap2d.tensor,
            offset=ap2d.offset + (fcnt - 1),
            ap=[[pstep, pcnt], [-1, fcnt]],
        )

    with tile.TileContext(nc) as tc:
        with (
            tc.tile_pool(name="consts", bufs=1) as consts,
            tc.tile_pool(name="xin", bufs=2) as xin,
            tc.tile_pool(name="h1p", bufs=3) as h1p,
            tc.tile_pool(name="ew", bufs=3) as ew,
            tc.tile_pool(name="yout", bufs=3) as yout,
            tc.tile_pool(name="psH", bufs=2, space="PSUM") as psH,
            tc.tile_pool(name="psZ", bufs=2, space="PSUM") as psZ,
            tc.tile_pool(name="psF", bufs=2, space="PSUM") as psF,
        ):
            w1_sb = consts.tile([C, 2 * C], f32r)
            nc.sync.dma_start(out=w1_sb, in_=w1catT[:, :])
            w2_sb = consts.tile([2 * C, 2 * C], mybir.dt.bfloat16)
            nc.sync.dma_start(out=w2_sb, in_=w2blk[:, :])
            b1_sb = consts.tile([2 * C, 1], fp32)
            nc.sync.dma_start(out=b1_sb, in_=b1cat[:, :])
            bz2_sb = consts.tile([2 * C, 1], fp32)
            nc.sync.dma_start(out=bz2_sb, in_=bz2p[:, :])
            bf2_sb = consts.tile([2 * C, 1], fp32)
            nc.sync.dma_start(out=bf2_sb, in_=bf2p[:, :])

            for b in range(B):
                for hp in range(HP):
                    x_t = xin.tile([C, T, S], f32r)
                    nc.sync.dma_start(
                        out=x_t,
                        in_=x[b, :, :, 2 * hp : 2 * hp + 2, :].rearrange(
                            "c t h w -> c t (h w)"
                        ),
                    )
                    # full-tile staging buffer in (t, w) order: rows
                    # [c0:32@h0, c0:32@h1, c32:64@h0, c32:64@h1]
                    y_ts = yout.tile([2 * C, T, W], fp32, tag="yts")
                    # 4 block-pairs: P = s[16j,16j+16) (h-row 0),
                    # Q = s[64+16j, 64+16j+16) (h-row 1)
                    for j in range(NBLK // 2):
                        pH = psH.tile([2 * C, 2, 512], fp32)
                        for q in range(2):
                            s0 = 64 * q + 16 * j
                            rhs = x_t[:, :, s0 : s0 + 16].rearrange(
                                "c t s -> c s t"
                            )
                            nc.tensor.matmul(
                                pH[:, q, :], w1_sb[:, :], rhs,
                                start=True, stop=True,
                            )
                        h1 = h1p.tile([2 * C, 2, 512], mybir.dt.bfloat16)
                        nc.scalar.activation(
                            h1.rearrange("p a n -> p (a n)"),
                            pH.rearrange("p a n -> p (a n)"),
                            TANH, bias=b1_sb[:, :],
                        )
                        pZ = psZ.tile([2 * C, 512], fp32)
                        pF = psF.tile([2 * C, 512], fp32)
                        # col-tiled L2 (f32r needs 64-aligned dst): pack the
                        # two blocks P,Q onto the 128 PSUM partitions:
                        # rows of psZ/psF = [P(ch 0:64), Q(ch 0:64)]
                        for q in range(2):
                            col = 64 * q
                            nc.tensor.matmul(
                                pZ[col : col + C, :],
                                w2_sb[0:C, 0:C], h1[0:C, q, :],
                                start=True, stop=True,
                                tile_position=(0, col),
                            )
                            nc.tensor.matmul(
                                pF[col : col + C, :],
                                w2_sb[C : 2 * C, C : 2 * C], h1[C : 2 * C, q, :],
                                start=True, stop=True,
                                tile_position=(64, col),
                            )
                        z_sb = ew.tile([2 * C, 16, 32], fp32, tag="z")
                        u_sb = ew.tile([2 * C, 16, 32], fp32, tag="u")
                        nc.scalar.activation(
                            z_sb.rearrange("p s t -> p (s t)"), pZ[:, :],
                            TANH, bias=bz2_sb[:, :],
                        )
                        nc.scalar.activation(
                            u_sb.rearrange("p s t -> p (s t)"), pF[:, :],
                            TANH, bias=bf2_sb[:, :],
                        )
                        u2 = u_sb.rearrange("p s t -> p (s t)")
                        z2 = z_sb.rearrange("p s t -> p (s t)")
                        a_sb = ew.tile([2 * C, 16, 32], fp32, tag="a")
                        g_sb = ew.tile([2 * C, 16, 32], fp32, tag="g")
                        a2 = a_sb.rearrange("p s t -> p (s t)")
                        g2 = g_sb.rearrange("p s t -> p (s t)")
                        # a = 0.5*u + 0.5 (= f) on GpSimd (DVE is the
                        # bottleneck engine; Pool is mostly idle)
                        nc.gpsimd.tensor_scalar(a2, u2, 0.5, 0.5, MUL, ADD)
                        # zero the coefficient at each segment's first
                        # step; fwd chans are rows [0:32] and [64:96],
                        # bwd chans rows [32:64] and [96:128]
                        nc.gpsimd.memset(a_sb[0:CH, :, 0], 0.0)
                        nc.gpsimd.memset(a_sb[CH:C, :, 31], 0.0)
                        nc.gpsimd.memset(a_sb[C : C + CH, :, 0], 0.0)
                        nc.gpsimd.memset(a_sb[C + CH :, :, 31], 0.0)
                        # g' = (u - 1) * z = -2*(1-f)*z; scan is linear in g
                        # so it yields -2*h, rescaled by -0.5 at relayout
                        nc.vector.scalar_tensor_tensor(
                            g2, u2, 1.0, z2, mybir.AluOpType.subtract, MUL
                        )
                        y_sb = yout.tile([2 * C, 16, 32], fp32, tag="ysb")
                        y2 = y_sb.rearrange("p s t -> p (s t)")
                        for r0 in (0, C):
                            nc.vector.tensor_tensor_scan(
                                y2[r0 : r0 + CH, :], a2[r0 : r0 + CH, :],
                                g2[r0 : r0 + CH, :], 0.0, MUL, ADD,
                            )
                            nc.vector.tensor_tensor_scan(
                                rev2d(y2[r0 + CH : r0 + C, :]),
                                rev2d(a2[r0 + CH : r0 + C, :]),
                                rev2d(g2[r0 + CH : r0 + C, :]),
                                0.0, MUL, ADD,
                            )
                        # relayout (s,t)->(t,s) + rescale by -0.5 on GpSimd
                        nc.gpsimd.tensor_scalar(
                            y_ts[:, :, 16 * j : 16 * j + 16],
                            y_sb.rearrange("p s t -> p t s"),
                            -0.5, None, MUL,
                        )
                    # store: 4 DMAs per tile, 256B-contiguous DRAM runs
                    # y_ts rows = [P ch0:32 | P ch32:64 | Q ch0:32 | Q ch32:64]
                    for q in range(2):
                        for half in range(2):
                            r0 = 64 * q + 32 * half
                            nc.sync.dma_start(
                                out=y[
                                    b,
                                    CH * half : CH * half + CH,
                                    :,
                                    2 * hp + q,
                                    :,
                                ],
                                in_=y_ts[r0 : r0 + 32, :, :],
                            )
    nc.compile()
    return nc


def _prep_weights(wz1, bz1, wz2, bz2, wf1, bf1, wf2, bf2):
    f32 = np.float32
    w1catT = np.ascontiguousarray(
        np.concatenate([wz1, wf1], axis=0).T, dtype=f32
    )  # [64, 128]
    import ml_dtypes
    w2blk = np.zeros((2 * C, 2 * C), dtype=f32)
    # rows 0:64 = h1z contraction, cols: [z_fwd(P-col 0:32 uses 0:32.. same
    # weights reused for Q via tile_position], layout: cols 0:32 z_fwd,
    # 32:64 z_bwd; rows 64:128 cols 64:96 f_fwd, 96:128 f_bwd (0.5-scaled)
    w2blk[0:C, 0:C] = wz2.T
    w2blk[C : 2 * C, C : C + CH] = 0.5 * wf2.T[:, 0:CH]
    w2blk[C : 2 * C, C + CH : 2 * C] = 0.5 * wf2.T[:, CH:C]
    w2blk = w2blk.astype(ml_dtypes.bfloat16)
    b1cat = np.concatenate([bz1, bf1]).astype(f32).reshape(-1, 1)
    # psZ rows = [zP_fwd(ch 0:32), zQ_fwd(ch 0:32), zP_bwd(ch 32:64), zQ_bwd]
    bz2p = np.concatenate([bz2, bz2]).astype(f32).reshape(-1, 1)
    bf2p = 0.5 * np.concatenate([bf2, bf2]).astype(f32).reshape(-1, 1)
    return dict(w1catT=w1catT, w2blk=w2blk, b1cat=b1cat, bz2p=bz2p, bf2p=bf2p)


def kernel(inputs, wz1, bz1, wz2, bz2, wf1, bf1, wf2, bf2):
    from concourse.bass_utils import run_bass_kernel_spmd

    if "nc" not in _built:
        _built["nc"] = _build()
    nc = _built["nc"]

    wd = _prep_weights(
        np.asarray(wz1), np.asarray(bz1), np.asarray(wz2), np.asarray(bz2),
        np.asarray(wf1), np.asarray(bf1), np.asarray(wf2), np.asarray(bf2),
    )
    xin = np.asarray(inputs, dtype=np.float32)
    in_maps = []
    for core in range(NCORES):
        shard = np.ascontiguousarray(xin[:, :, :, core * HL : (core + 1) * HL, :])
        m = {"x": shard}
        m.update(wd)
        in_maps.append(m)

    res = run_bass_kernel_spmd(nc, in_maps, core_ids=list(range(NCORES)))
    out = np.concatenate([r["y"] for r in res.results], axis=3)
    return out



# revision 2
# speedup vs baseline: 3.0232x; 3.0232x over previous
"""Trainium2 Bass kernel for bidirectional gated linear recurrence block, v6.

Reference computation (per spatial position, channel-mixing MLPs):
  Z = tanh(W_z2 @ tanh(W_z1 @ x + b_z1) + b_z2)
  F = sigmoid(W_f2 @ tanh(W_f1 @ x + b_f1) + b_f2)
  channels 0:32  : h_t = F*h_{t-1} + (1-F)*Z forward over T
  channels 32:64 : same recurrence backward over T

Sharding: H (=64) split across 8 cores, 8 rows each; no collectives.

v6 = v5 + warmup/drain trims: per-h-pair x/y DMAs, segment-start memset
on DVE, and a dummy sigmoid up front so the combined tanh+sigmoid ACT
table loads once during warmup.

v5 core: explicit software pipelining. Per 512-position block k the work is
  PE : L1(k) 2 matmuls (427ns) + L2(k) 8 narrow matmuls (1707ns)
  ACT: h1(k) tanh [128,1024] (997ns) + z(k)/F(k) [128,512] (612ns each)
and the two engines have near-equal totals (~2.2us/block), so the emit
order skews L1 one block ahead and orders L2 z-chain first:
  PE queue : L1(k), L2(k-1)   |  ACT queue: z(k-1), F(k-1), h1(k)
which lets each engine run back-to-back instead of ping-ponging.

Other structure (from v4):
  - bf16 input [B, HH, C, 2, T, 2, W]; one 16KB-row DMA per h-quad.
  - L2 bwd-channel matmuls read h1 through time-REVERSED rhs APs so all
    post-L2 stages are direction-uniform: ONE 128-partition scan per pair.
    Host un-reverses bwd time at unshard.
  - z weights/bias negated -> ACT emits -z; g = (F-1)(-z) = (1-F)z in one
    scalar_tensor_tensor. Gate a = F from Sigmoid (same ACT table as Tanh);
    one memset zeroes segment starts.
  - PSUM: pH x2 + pZ x2 + pF x2 = 8 banks, all double-buffered.
  - bf16 output in scan layout; one DMA per h-quad on the Pool queue.
"""

import numpy as np

B, C, T, H, W = 2, 64, 32, 64, 64
NCORES = 8
HL = H // NCORES          # 8 h-rows per core
HP = HL // 2              # 4 h-pair tiles per batch entry
HH = HP // 2              # 2 h-quads per batch entry
S = 2 * W                 # 128 positions per h-pair (h2, w)
NJ = 4                    # 4 block-pairs per h-pair
CH = C // 2               # 32 = fwd (or bwd) channel count

_built = {}


def _build():
    import concourse.bass as bass
    import concourse.mybir as mybir
    import concourse.tile as tile
    from concourse import bacc

    fp32 = mybir.dt.float32
    bf16 = mybir.dt.bfloat16
    nc = bacc.Bacc(None, target_bir_lowering=False)

    x = nc.dram_tensor("x", [B, HH, C, 2, T, 2, W], bf16, kind="ExternalInput")
    wall = nc.dram_tensor("wall", [3 * C, 2 * C], bf16, kind="ExternalInput")
    bias = nc.dram_tensor("bias", [2 * C, 3], fp32, kind="ExternalInput")
    y = nc.dram_tensor(
        "y", [B, HH, 2 * C, 2, NJ * 512], bf16, kind="ExternalOutput"
    )

    MUL = mybir.AluOpType.mult
    ADD = mybir.AluOpType.add
    SUB = mybir.AluOpType.subtract
    TANH = mybir.ActivationFunctionType.Tanh
    SIGM = mybir.ActivationFunctionType.Sigmoid

    def rev_t(ap2d, tlen):
        # [P, F] contiguous AP -> [P, seg, t] with innermost t reversed.
        (pstep, pcnt), (fstep, fcnt) = [list(d) for d in ap2d.ap]
        assert fstep == 1 and fcnt % tlen == 0, ap2d.ap
        return bass.AP(
            tensor=ap2d.tensor,
            offset=ap2d.offset + (tlen - 1),
            ap=[[pstep, pcnt], [tlen, fcnt // tlen], [-1, tlen]],
        )

    with tile.TileContext(nc) as tc:
        with (
            tc.tile_pool(name="consts", bufs=1) as consts,
            tc.tile_pool(name="xin", bufs=2) as xin,
            tc.tile_pool(name="h1p", bufs=3) as h1p,
            tc.tile_pool(name="ew", bufs=2) as ew,
            tc.tile_pool(name="yout", bufs=2) as yout,
            tc.tile_pool(name="psH", bufs=2, space="PSUM") as psH,
            tc.tile_pool(name="psZ", bufs=2, space="PSUM") as psZ,
            tc.tile_pool(name="psF", bufs=2, space="PSUM") as psF,
        ):
            w1_sb = consts.tile([C, 2 * C], bf16)
            nc.sync.dma_start(out=w1_sb, in_=wall[0:C, :])
            w2_sb = consts.tile([2 * C, C], bf16)
            nc.sync.dma_start(out=w2_sb, in_=wall[C : 3 * C, 0:C])
            b_sb = consts.tile([2 * C, 3], fp32)
            nc.sync.dma_start(out=b_sb, in_=bias[:, :])
            # dummy sigmoid pins the tanh+sigmoid table load into warmup
            warm = consts.tile([2 * C, 1], bf16)
            nc.scalar.activation(warm, b_sb[:, 0:1], SIGM)

            # flat block list: (b, hh, u, jj, j2); 512 positions each
            blocks = [
                (b, hh, u, jj, j2)
                for b in range(B)
                for hh in range(HH)
                for u in range(2)
                for jj in range(NJ // 2)
                for j2 in range(2)
            ]
            N = len(blocks)
            quads = {}   # (b, hh, u) -> dict(x_t, y_t)
            st = {}      # k -> per-block tiles
            pairs = {}   # (b, hh, u, jj) -> dict(z, a, g)

            def emit_L1(k):
                b, hh, u, jj, j2 = blocks[k]
                qd = quads.get((b, hh, u))
                if qd is None:
                    x_t = xin.tile([C, T, S], bf16)
                    nc.sync.dma_start(
                        out=x_t,
                        in_=x[b, hh, :, u].rearrange("c t r w -> c t (r w)"),
                    )
                    y_t = yout.tile([2 * C, NJ, 512], bf16, tag="yt")
                    qd = quads[(b, hh, u)] = dict(x_t=x_t, y_t=y_t)
                pH = psH.tile([2 * C, 2, 512], fp32)
                j = 2 * jj + j2
                for q in range(2):
                    s0 = 64 * q + 16 * j
                    rhs = qd["x_t"][:, :, s0 : s0 + 16].rearrange(
                        "c t s -> c s t"
                    )
                    nc.tensor.matmul(
                        pH[:, q, :], w1_sb[:, :], rhs, start=True, stop=True
                    )
                st[k] = dict(pH=pH)

            def emit_h1(k):
                pH = st[k].pop("pH")
                h1 = h1p.tile([2 * C, 2, 512], bf16)
                nc.scalar.activation(
                    h1.rearrange("p a n -> p (a n)"),
                    pH.rearrange("p a n -> p (a n)"),
                    TANH, bias=b_sb[:, 0:1],
                )
                st[k]["h1"] = h1

            def emit_L2(k):
                h1 = st[k].pop("h1")
                pZ = psZ.tile([2 * C, 512], fp32)
                pF = psF.tile([2 * C, 512], fp32)
                # z-chain first so the z activation unblocks at 50% of L2
                for q in range(2):
                    col = 64 * q
                    hz = h1[0:C, q, :]
                    nc.tensor.matmul(
                        pZ[col : col + CH, :], w2_sb[0:C, 0:CH], hz,
                        start=True, stop=True, tile_position=(0, col),
                    )
                    nc.tensor.matmul(
                        pZ[col + CH : col + C, :], w2_sb[0:C, CH:C],
                        rev_t(hz, T),
                        start=True, stop=True, tile_position=(0, col + CH),
                    )
                for q in range(2):
                    col = 64 * q
                    hf = h1[C : 2 * C, q, :]
                    nc.tensor.matmul(
                        pF[col : col + CH, :], w2_sb[C : 2 * C, 0:CH], hf,
                        start=True, stop=True, tile_position=(64, col),
                    )
                    nc.tensor.matmul(
                        pF[col + CH : col + C, :], w2_sb[C : 2 * C, CH:C],
                        rev_t(hf, T),
                        start=True, stop=True, tile_position=(64, col + CH),
                    )
                st[k]["pZ"] = pZ
                st[k]["pF"] = pF

            def emit_zf(k):
                b, hh, u, jj, j2 = blocks[k]
                pr = pairs.get((b, hh, u, jj))
                if pr is None:
                    pr = pairs[(b, hh, u, jj)] = dict(
                        z=ew.tile([2 * C, 2, 16, 32], bf16, tag="z", name="z_sb"),
                        a=ew.tile([2 * C, 2, 16, 32], bf16, tag="a", name="a_sb"),
                        g=ew.tile([2 * C, 2, 16, 32], bf16, tag="g", name="g_sb"),
                    )
                pZ = st[k].pop("pZ")
                pF = st[k].pop("pF")
                nc.scalar.activation(
                    pr["z"][:, j2].rearrange("p s t -> p (s t)"),
                    pZ[:, :], TANH, bias=b_sb[:, 1:2],
                )
                nc.scalar.activation(
                    pr["a"][:, j2].rearrange("p s t -> p (s t)"),
                    pF[:, :], SIGM, bias=b_sb[:, 2:3],
                )
                del st[k]

            def emit_epi(k, final=False):
                b, hh, u, jj, j2 = blocks[k]
                pr = pairs.get((b, hh, u, jj))
                qd = quads[(b, hh, u)]
                if final:
                    # drain trim: per-block gate+scan on the last pair
                    sl = slice(j2, j2 + 1)
                    z2 = pr["z"][:, sl].rearrange("p a s t -> p (a s t)")
                    a2 = pr["a"][:, sl].rearrange("p a s t -> p (a s t)")
                    g2 = pr["g"][:, sl].rearrange("p a s t -> p (a s t)")
                    nc.vector.scalar_tensor_tensor(g2, a2, 1.0, z2, SUB, MUL)
                    nc.vector.memset(pr["a"][:, sl, :, 0], 0.0)
                    ysl = slice(2 * jj + j2, 2 * jj + j2 + 1)
                    yj = qd["y_t"][:, ysl].rearrange("p j n -> p (j n)")
                    nc.vector.tensor_tensor_scan(yj, a2, g2, 0.0, MUL, ADD)
                    if j2 != 1:
                        return
                elif j2 != 1:
                    return
                else:
                    z2 = pr["z"].rearrange("p a s t -> p (a s t)")
                    a2 = pr["a"].rearrange("p a s t -> p (a s t)")
                    g2 = pr["g"].rearrange("p a s t -> p (a s t)")
                    nc.vector.scalar_tensor_tensor(g2, a2, 1.0, z2, SUB, MUL)
                    nc.vector.memset(pr["a"][:, :, :, 0], 0.0)
                    yjj = qd["y_t"][:, 2 * jj : 2 * jj + 2].rearrange(
                        "p a n -> p (a n)"
                    )
                    nc.vector.tensor_tensor_scan(yjj, a2, g2, 0.0, MUL, ADD)
                pairs.pop((b, hh, u, jj))
                if jj == NJ // 2 - 1:
                    nc.gpsimd.dma_start(
                        out=y[b, hh, :, u],
                        in_=qd.pop("y_t").rearrange("p j n -> p (j n)"),
                    )
                    del quads[(b, hh, u)]

            # software-pipelined emission (skew 1):
            #   PE : L1(k), L2(k-1) | ACT: z(k-1), F(k-1), h1(k)
            emit_L1(0)
            emit_h1(0)
            for k in range(1, N):
                emit_L1(k)
                emit_L2(k - 1)
                emit_zf(k - 1)
                emit_epi(k - 1, final=(k - 1 == N - 2))
                emit_h1(k)
            emit_L2(N - 1)
            emit_zf(N - 1)
            emit_epi(N - 1, final=True)
    nc.compile()
    return nc


def _prep_weights(wz1, bz1, wz2, bz2, wf1, bf1, wf2, bf2):
    import ml_dtypes

    f32 = np.float32
    wall = np.zeros((3 * C, 2 * C), dtype=f32)
    wall[0:C, :] = np.concatenate([wz1, wf1], axis=0).T  # L1 lhsT
    wall[C : 2 * C, 0:C] = -wz2.T                        # negated: tanh -> -z
    wall[2 * C : 3 * C, 0:C] = wf2.T
    wall = wall.astype(ml_dtypes.bfloat16)
    bias = np.stack(
        [
            np.concatenate([bz1, bf1]),
            np.concatenate([-bz2, -bz2]),
            np.concatenate([bf2, bf2]),
        ],
        axis=1,
    ).astype(f32)
    return dict(wall=wall, bias=bias)


def _prep_x(xin):
    """Full fp32 input -> per-core bf16 [B, HH, C, 2, T, 2, W] shards."""
    import ml_dtypes

    xb = np.asarray(xin, dtype=np.float32).astype(ml_dtypes.bfloat16)
    xr = xb.reshape(B, C, T, NCORES, HH, 2, 2, W)
    return [
        np.ascontiguousarray(xr[:, :, :, core].transpose(0, 3, 1, 4, 2, 5, 6))
        for core in range(NCORES)
    ]


def _unshard_y(results):
    """Per-core bf16 [B, HH, 128, 2, 2048] -> full fp32 [B, C, T, H, W].

    Row r = 64q + 32d + c (q = h-row in pair, d = direction); free
    (u = h-pair in quad, j, s, t); bwd rows (d=1) carry time reversed.
    """
    outs = []
    for r in results:
        buf = np.asarray(r["y"], dtype=np.float32).reshape(
            B, HH, 2, 2, CH, 2, NJ, 16, T
        )
        fwd = buf[:, :, :, 0]
        bwd = buf[:, :, :, 1, :, :, :, :, ::-1]
        st = np.stack([fwd, bwd], axis=3)  # [B, HH, q, d, c, u, j, s, t]
        o = st.transpose(0, 3, 4, 8, 1, 5, 2, 6, 7).reshape(B, C, T, HL, W)
        outs.append(o)
    return np.concatenate(outs, axis=3)


def kernel(inputs, wz1, bz1, wz2, bz2, wf1, bf1, wf2, bf2):
    from concourse.bass_utils import run_bass_kernel_spmd

    if "nc" not in _built:
        _built["nc"] = _build()
    nc = _built["nc"]

    wd = _prep_weights(
        np.asarray(wz1), np.asarray(bz1), np.asarray(wz2), np.asarray(bz2),
        np.asarray(wf1), np.asarray(bf1), np.asarray(wf2), np.asarray(bf2),
    )
    in_maps = []
    for shard in _prep_x(inputs):
        m = {"x": shard}
        m.update(wd)
        in_maps.append(m)

    res = run_bass_kernel_spmd(nc, in_maps, core_ids=list(range(NCORES)))
    return _unshard_y(res.results)
